# revision 18
# baseline (speedup 1.0000x reference)
"""TRN2 kernel for nn_BClassifier: attention-MIL pooling on 8 NeuronCores.

Heavy stage (memory-bound): x [4, 20000, 512] attention pooling.
Sharding: 2 cores per bag, ~10000 instance rows each (zero-padded to 10240
so every tile is 128 rows). Each core computes exp partials; per-row e is
exported so the host sums only valid rows. Host combines into the pooled
bag embedding M. The tiny 256-node hypergraph stage runs on host.

V3: f32->bf16 cast during DMA load (SWDGE), X-bar DMA transposes instead of
PE transposes, 1MB block loads, h1 matmuls stream N=512.
"""
import sys
import numpy as np

sys.path.insert(0, "/opt/trn_rl_repo")

from concourse import bass, bacc, mybir, tile  # noqa: E402
from concourse import bass_utils  # noqa: E402

F32 = mybir.dt.float32
BF16 = mybir.dt.bfloat16
AF = mybir.ActivationFunctionType

B, N, D, C = 4, 20000, 512, 4
NUM_NODE = 256
K = 8
EPS_GN = 1e-5
N_CORES = 8
SUB = 128                       # rows per partition tile
NSUB = 4                        # sub-tiles per block
BLK = SUB * NSUB                # 512 rows per block
NBLK = 20
R_PAD = BLK * NBLK              # 10240 padded rows per core
# valid rows per core: even cores 9984, odd cores 10016 (bag = 2 cores)
R_EVEN, R_ODD = 9984, 10016


def _build_mil_program():
    nc = bacc.Bacc(
        "TRN2",
        target_bir_lowering=False,
        debug=False,
        enable_asserts=False,
        num_devices=N_CORES,
    )
    x_d = nc.dram_tensor("x", [R_PAD, D], F32, kind="ExternalInput")
    aW1_d = nc.dram_tensor("aW1", [D, D], F32, kind="ExternalInput")
    ab1_d = nc.dram_tensor("ab1", [D], F32, kind="ExternalInput")
    aW2_d = nc.dram_tensor("aW2", [D, 1], F32, kind="ExternalInput")
    ab2_d = nc.dram_tensor("ab2", [1], F32, kind="ExternalInput")
    m_d = nc.dram_tensor("m_part", [1, D], F32, kind="ExternalOutput")
    e_d = nc.dram_tensor("e_part", [1, R_PAD], F32, kind="ExternalOutput")

    with tile.TileContext(nc) as tc:
        with (
            tc.tile_pool(name="const", bufs=1) as cpool,
            tc.tile_pool(name="io", bufs=2) as iopool,
            tc.tile_pool(name="xt", bufs=2) as xtpool,
            tc.tile_pool(name="h1", bufs=2) as h1pool,
            tc.tile_pool(name="ev", bufs=2) as evpool,
            tc.tile_pool(name="ps", bufs=4, space=bass.MemorySpace.PSUM) as pspool,
            tc.tile_pool(name="pssm", bufs=2, space=bass.MemorySpace.PSUM) as pssmpool,
            tc.tile_pool(name="acc", bufs=1, space=bass.MemorySpace.PSUM) as accpool,
        ):
            one_bf = cpool.tile([1, 1], BF16)
            nc.gpsimd.memset(one_bf[:], 1.0)

            # weights, cast to bf16 during DMA (SWDGE)
            aW1_sb = cpool.tile([128, 4 * D], BF16)
            for kc in range(4):
                nc.gpsimd.dma_start(
                    aW1_sb[:, kc * D:(kc + 1) * D], aW1_d[kc * 128:(kc + 1) * 128, :]
                )
            ab1_sb = cpool.tile([128, 4], F32)
            nc.sync.dma_start(ab1_sb[:, :], ab1_d.ap().rearrange("(a p) -> p a", p=128))
            aW2_sb = cpool.tile([128, 4], BF16)
            nc.gpsimd.dma_start(aW2_sb[:, :], aW2_d.ap().rearrange("(a p) o -> p (a o)", p=128))
            ab2_sb = cpool.tile([1, 1], F32)
            nc.sync.dma_start(ab2_sb[:, :], ab2_d.ap().rearrange("(a b) -> a b", a=1))

            m_acc = accpool.tile([1, D], F32)

            for blk in range(NBLK):
                r0 = blk * BLK
                # one 1MB cast-DMA: xb[p, t*512+d] = x[r0 + t*128 + p, d] (bf16)
                xb = iopool.tile([128, NSUB * D], BF16, tag="xb")
                nc.gpsimd.dma_start(
                    xb[:],
                    x_d[r0:r0 + BLK, :].rearrange("(t p) d -> p t d", p=128),
                )

                # X-bar transpose per sub-tile: xT[p, kc*512 + t*128 + r]
                #   = feature (kc*128+p) of row (t*128+r)
                xT_sb = xtpool.tile([128, 4 * BLK], BF16, tag="xt_sb")
                xT_4d = xT_sb[:].rearrange("p (c s r) -> p c s r", c=4, s=NSUB)
                for t in range(NSUB):
                    nc.sync.dma_start(
                        xT_4d[:, :, t, :],
                        xb[:, t * D:(t + 1) * D],
                        transpose=True,
                    )

                # h1T = relu(aW1.T @ xT + ab1): [4 mo x 128 outfeat, 512 rows] bf16
                h1T_sb = h1pool.tile([128, 4 * BLK], BF16, tag="h1t")
                for mo in range(4):
                    h1_ps = pspool.tile([128, BLK], F32, tag="h1_ps")
                    for kc in range(4):
                        nc.tensor.matmul(
                            h1_ps[:],
                            aW1_sb[:, kc * D + mo * 128: kc * D + (mo + 1) * 128],
                            xT_sb[:, kc * BLK:(kc + 1) * BLK],
                            start=(kc == 0),
                            stop=(kc == 3),
                        )
                    nc.scalar.activation(
                        h1T_sb[:, mo * BLK:(mo + 1) * BLK],
                        h1_ps[:],
                        AF.Relu,
                        bias=ab1_sb[:, mo:mo + 1],
                    )

                # h2 = aW2.T @ h1T -> [1, 512]; e = exp(h2 + ab2)
                h2_ps = pssmpool.tile([1, BLK], F32, tag="small_ps")
                for mo in range(4):
                    nc.tensor.matmul(
                        h2_ps[:],
                        aW2_sb[:, mo:mo + 1],
                        h1T_sb[:, mo * BLK:(mo + 1) * BLK],
                        start=(mo == 0),
                        stop=(mo == 3),
                    )
                e_sb = evpool.tile([1, BLK], F32, tag="e")
                nc.scalar.activation(
                    e_sb[:], h2_ps[:], AF.Exp, bias=ab2_sb[0:1, 0:1],
                )
                nc.scalar.dma_start(e_d[0:1, r0:r0 + BLK], e_sb[:])
                e_bf = evpool.tile([1, BLK], BF16, tag="e_bf")
                nc.vector.tensor_copy(e_bf[:], e_sb[:])

                # eT [128, 4] via K=1 matmuls, cast to bf16
                eT_ps = pssmpool.tile([SUB, NSUB], F32, tag="small_ps")
                for t in range(NSUB):
                    nc.tensor.matmul(
                        eT_ps[:, t:t + 1],
                        e_bf[:, t * SUB:(t + 1) * SUB],
                        one_bf[:],
                        start=True, stop=True,
                    )
                eT_bf = evpool.tile([SUB, NSUB], BF16, tag="eT_bf")
                nc.vector.tensor_copy(eT_bf[:], eT_ps[:])

                # m_part += e_t @ x_t for each sub-tile
                for t in range(NSUB):
                    nc.tensor.matmul(
                        m_acc[:], eT_bf[:, t:t + 1], xb[:, t * D:(t + 1) * D],
                        start=(blk == 0 and t == 0),
                        stop=(blk == NBLK - 1 and t == NSUB - 1),
                        skip_group_check=True,
                    )

            m_out_sb = cpool.tile([1, D], F32)
            nc.any.tensor_copy(out=m_out_sb[:], in_=m_acc[:])
            nc.sync.dma_start(m_d[:, :], m_out_sb[:])

    nc.compile()
    return nc


_NC_CACHE = {}


def _get_mil_program():
    if "mil" not in _NC_CACHE:
        _NC_CACHE["mil"] = _build_mil_program()
    return _NC_CACHE["mil"]


def _make_in_maps(x, aW1, ab1, aW2, ab2):
    in_maps = []
    for c in range(N_CORES):
        b, half = c // 2, c % 2
        nvalid = R_EVEN if half == 0 else R_ODD
        lo = 0 if half == 0 else R_EVEN
        xs = np.zeros((R_PAD, D), np.float32)
        xs[:nvalid] = x[b, lo:lo + nvalid]
        in_maps.append({
            "x": xs,
            "aW1": np.ascontiguousarray(aW1, dtype=np.float32),
            "ab1": np.ascontiguousarray(ab1, dtype=np.float32),
            "aW2": np.ascontiguousarray(aW2, dtype=np.float32),
            "ab2": np.ascontiguousarray(ab2, dtype=np.float32),
        })
    return in_maps


def _lrelu(x, s=0.01):
    return np.where(x >= 0, x, s * x)


def _graph_norm(x, w, b, ms):
    mean = x.mean(axis=0)
    out = x - mean * ms
    var = (out * out).mean(axis=0)
    return w * out / np.sqrt(var + EPS_GN) + b


def _hypergraph_conv_dense(x, he_attr, S, W, att, bias):
    """Dense form of PyG HypergraphConv (heads=1, attention) on the kNN
    hypergraph. S[i, j] = 1 iff j in nbr[i]; hyperedge i has the K nbrs of
    node i. Edge (dst=i, src=j) exists iff S[i,j]=1."""
    xw = x @ W                       # [256, F]
    hw = he_attr @ W                 # [256, F]
    d = att.shape[0] // 2
    p = xw @ att[:d]                 # [256] (src term)
    q = hw @ att[d:]                 # [256] (dst term)
    A = _lrelu(q[:, None] + p[None, :], 0.2)     # [i, j]
    mask = S > 0
    neg = np.where(mask, A, -np.inf)
    col_max = neg.max(axis=0)                    # per src node j
    col_max = np.where(np.isfinite(col_max), col_max, 0.0)
    E = np.where(mask, np.exp(A - col_max[None, :]), 0.0)
    colsum = E.sum(axis=0)
    colsum = np.where(colsum > 0, colsum, 1.0)
    alpha = E / colsum[None, :]
    deg = S.sum(axis=0)                          # node degree D (as src)
    Dinv = np.where(deg > 0, 1.0 / deg, 0.0)
    edge_feat = (alpha / K) @ xw                 # node -> hyperedge
    out = Dinv[:, None] * (alpha.T @ edge_feat)  # hyperedge -> node
    return out + bias


def _graph_stage(M, rehearsal, dW1, db1, dW2, db2,
                 h1W, h1att, h1b, h2W, h2att, h2b,
                 n1w, n1b, n1ms, n2w, n2b, n2ms,
                 f1W, f1b, f2W, f2b, gW1, gb1, gW2, gb2, cW, cb):
    x_concat = np.concatenate([M, rehearsal], axis=0)[:NUM_NODE]
    xf = _lrelu(_lrelu(x_concat @ dW1 + db1) @ dW2 + db2)
    norms = np.maximum(np.linalg.norm(xf, axis=1, keepdims=True), 1e-12)
    xn = xf / norms
    sim = xn @ xn.T
    # top-K per row via threshold on the 8th largest value
    kth = np.partition(sim, NUM_NODE - K, axis=1)[:, NUM_NODE - K]
    S = (sim >= kth[:, None]).astype(np.float32)
    edge_attr = (S @ xf) / K

    g1 = _hypergraph_conv_dense(xf, edge_attr, S, h1W, h1att, h1b)
    g1 = _lrelu(_graph_norm(g1, n1w, n1b, n1ms))
    out1 = _lrelu(g1 @ f1W + f1b)
    g2 = _hypergraph_conv_dense(g1, edge_attr, S, h2W, h2att, h2b)
    g2 = _lrelu(_graph_norm(g2, n2w, n2b, n2ms))
    out2 = _lrelu(g2 @ f2W + f2b)

    out = np.concatenate([xf, out1, out2], axis=1)   # [256, 1024]
    s = np.maximum(out.T @ gW1 + gb1, 0.0) @ gW2 + gb2
    s = 1.0 / (1.0 + np.exp(-s))
    s = s[:, 0] - np.mean(s)
    logits = (out * s[None, :]) @ cW + cb
    return logits


def kernel(**inputs):
    inp = {k: np.asarray(v) for k, v in inputs.items()}
    x = inp["x"].astype(np.float32)

    nc = _get_mil_program()
    in_maps = _make_in_maps(x, inp["aW1"], inp["ab1"], inp["aW2"], inp["ab2"])
    res = bass_utils.run_bass_kernel_spmd(nc, in_maps, core_ids=list(range(N_CORES)))

    m = np.stack([res.results[c]["m_part"][0] for c in range(N_CORES)])  # [8, 512]
    s = np.array([
        res.results[c]["e_part"][0, :(R_EVEN if c % 2 == 0 else R_ODD)].sum()
        for c in range(N_CORES)
    ])
    M = np.stack([(m[2 * b] + m[2 * b + 1]) / (s[2 * b] + s[2 * b + 1])
                  for b in range(B)]).astype(np.float32)                 # [4, 512]

    logits_mlp = (M @ inp["bagW"] + inp["bagB"]).astype(np.float32)

    gkeys = ["rehearsal", "dW1", "db1", "dW2", "db2",
             "h1W", "h1att", "h1b", "h2W", "h2att", "h2b",
             "n1w", "n1b", "n1ms", "n2w", "n2b", "n2ms",
             "f1W", "f1b", "f2W", "f2b", "gW1", "gb1", "gW2", "gb2", "cW", "cb"]
    logits = _graph_stage(M, *[inp[k].astype(np.float32) for k in gkeys])
    logits_graph = logits[:B].astype(np.float32)
    return logits_mlp, logits_graph


# revision 19
# speedup vs baseline: 2.0456x; 2.0456x over previous
"""TRN2 kernel for nn_BClassifier: attention-MIL pooling on 8 NeuronCores.

Heavy stage (memory-bound): x [4, 20000, 512] attention pooling.
Sharding: 2 cores per bag, ~10000 instance rows each (zero-padded to 10240
so every tile is 128 rows). The host ships each core's shard twice in bf16,
block-major: natural layout (for the sum_r e_r*x_r reduction, which
contracts rows) and feature-major (for the h = relu(x@aW1)@aW2 chain, which
contracts features) -- same total bytes as one f32 copy, and the device
needs no transposes or casts. Each core computes exp partials; per-row e is
exported so the host sums only valid rows and combines partials into the
pooled bag embedding M. The tiny 256-node hypergraph stage runs on host.
"""
import sys
import numpy as np
import ml_dtypes

sys.path.insert(0, "/opt/trn_rl_repo")

from concourse import bass, bacc, mybir, tile  # noqa: E402
from concourse import bass_utils  # noqa: E402

F32 = mybir.dt.float32
BF16 = mybir.dt.bfloat16
AF = mybir.ActivationFunctionType
BF = ml_dtypes.bfloat16

B, N, D, C = 4, 20000, 512, 4
NUM_NODE = 256
K = 8
EPS_GN = 1e-5
N_CORES = 8
SUB = 128                       # rows per partition tile
NSUB = 4                        # sub-tiles per block
BLK = SUB * NSUB                # 512 rows per block
NBLK = 20
R_PAD = BLK * NBLK              # 10240 padded rows per core
# valid rows per core: even cores 9984, odd cores 10016 (bag = 2 cores)
R_EVEN, R_ODD = 9984, 10016


def _build_mil_program():
    nc = bacc.Bacc(
        "TRN2",
        target_bir_lowering=False,
        debug=False,
        enable_asserts=False,
        num_devices=N_CORES,
    )
    # xn[blk, p, t, d] = x[blk*512 + t*128 + p, d]           (natural, bf16)
    xn_d = nc.dram_tensor("xn", [NBLK, 128, NSUB, D], BF16, kind="ExternalInput")
    # xt[blk, p, c, r] = x[blk*512 + r, c*128 + p]           (feature-major, bf16)
    xt_d = nc.dram_tensor("xt", [NBLK, 128, 4, BLK], BF16, kind="ExternalInput")
    aW1_d = nc.dram_tensor("aW1", [128, 4 * D], BF16, kind="ExternalInput")
    ab1_d = nc.dram_tensor("ab1", [128, 4], F32, kind="ExternalInput")
    aW2_d = nc.dram_tensor("aW2", [128, 4], BF16, kind="ExternalInput")
    ab2_d = nc.dram_tensor("ab2", [1, 1], F32, kind="ExternalInput")
    m_d = nc.dram_tensor("m_part", [1, D], F32, kind="ExternalOutput")
    e_d = nc.dram_tensor("e_part", [1, R_PAD], F32, kind="ExternalOutput")

    with tile.TileContext(nc) as tc:
        with (
            tc.tile_pool(name="const", bufs=1) as cpool,
            tc.tile_pool(name="io", bufs=3) as iopool,
            tc.tile_pool(name="h1", bufs=2) as h1pool,
            tc.tile_pool(name="ev", bufs=2) as evpool,
            tc.tile_pool(name="ps", bufs=4, space=bass.MemorySpace.PSUM) as pspool,
            tc.tile_pool(name="pssm", bufs=2, space=bass.MemorySpace.PSUM) as pssmpool,
            tc.tile_pool(name="acc", bufs=1, space=bass.MemorySpace.PSUM) as accpool,
        ):
            one_bf = cpool.tile([1, 1], BF16)
            nc.gpsimd.memset(one_bf[:], 1.0)

            aW1_sb = cpool.tile([128, 4 * D], BF16)
            nc.sync.dma_start(aW1_sb[:], aW1_d[:, :])
            ab1_sb = cpool.tile([128, 4], F32)
            nc.sync.dma_start(ab1_sb[:], ab1_d[:, :])
            aW2_sb = cpool.tile([128, 4], BF16)
            nc.sync.dma_start(aW2_sb[:], aW2_d[:, :])
            ab2_sb = cpool.tile([1, 1], F32)
            nc.sync.dma_start(ab2_sb[:], ab2_d[:, :])

            m_acc = accpool.tile([1, D], F32)

            for blk in range(NBLK):
                r0 = blk * BLK
                xn_sb = iopool.tile([128, NSUB * D], BF16, tag="xn")
                nc.sync.dma_start(
                    xn_sb[:], xn_d[blk].rearrange("p t d -> p (t d)"))
                xT_sb = iopool.tile([128, 4 * BLK], BF16, tag="xt")
                nc.scalar.dma_start(
                    xT_sb[:], xt_d[blk].rearrange("p c r -> p (c r)"))

                # h1T = relu(aW1.T @ xT + ab1): [4 mo x 128 outfeat, 512 rows]
                h1T_sb = h1pool.tile([128, 4 * BLK], BF16, tag="h1t")
                for mo in range(4):
                    h1_ps = pspool.tile([128, BLK], F32, tag="h1_ps")
                    for kc in range(4):
                        nc.tensor.matmul(
                            h1_ps[:],
                            aW1_sb[:, kc * D + mo * 128: kc * D + (mo + 1) * 128],
                            xT_sb[:, kc * BLK:(kc + 1) * BLK],
                            start=(kc == 0),
                            stop=(kc == 3),
                        )
                    nc.scalar.activation(
                        h1T_sb[:, mo * BLK:(mo + 1) * BLK],
                        h1_ps[:],
                        AF.Relu,
                        bias=ab1_sb[:, mo:mo + 1],
                    )

                # h2 = aW2.T @ h1T -> [1, 512]; e = exp(h2 + ab2)
                h2_ps = pssmpool.tile([1, BLK], F32, tag="small_ps")
                for mo in range(4):
                    nc.tensor.matmul(
                        h2_ps[:],
                        aW2_sb[:, mo:mo + 1],
                        h1T_sb[:, mo * BLK:(mo + 1) * BLK],
                        start=(mo == 0),
                        stop=(mo == 3),
                    )
                e_sb = evpool.tile([1, BLK], F32, tag="e")
                nc.scalar.activation(
                    e_sb[:], h2_ps[:], AF.Exp, bias=ab2_sb[0:1, 0:1],
                )
                nc.gpsimd.dma_start(e_d[0:1, r0:r0 + BLK], e_sb[:])
                e_bf = evpool.tile([1, BLK], BF16, tag="e_bf")
                nc.vector.tensor_copy(e_bf[:], e_sb[:])

                # eT [128, 4] via K=1 matmuls, cast to bf16
                eT_ps = pssmpool.tile([SUB, NSUB], F32, tag="small_ps")
                for t in range(NSUB):
                    nc.tensor.matmul(
                        eT_ps[:, t:t + 1],
                        e_bf[:, t * SUB:(t + 1) * SUB],
                        one_bf[:],
                        start=True, stop=True,
                    )
                eT_bf = evpool.tile([SUB, NSUB], BF16, tag="eT_bf")
                nc.vector.tensor_copy(eT_bf[:], eT_ps[:])

                # m_part += e_t @ x_t for each sub-tile
                for t in range(NSUB):
                    nc.tensor.matmul(
                        m_acc[:], eT_bf[:, t:t + 1], xn_sb[:, t * D:(t + 1) * D],
                        start=(blk == 0 and t == 0),
                        stop=(blk == NBLK - 1 and t == NSUB - 1),
                        skip_group_check=True,
                    )

            m_out_sb = cpool.tile([1, D], F32)
            nc.any.tensor_copy(out=m_out_sb[:], in_=m_acc[:])
            nc.gpsimd.dma_start(m_d[:, :], m_out_sb[:])

    nc.compile()
    return nc


_NC_CACHE = {}


def _get_mil_program():
    if "mil" not in _NC_CACHE:
        _NC_CACHE["mil"] = _build_mil_program()
    return _NC_CACHE["mil"]


def _make_in_maps(x, aW1, ab1, aW2, ab2):
    aW1 = np.asarray(aW1, np.float32)
    ab1 = np.asarray(ab1, np.float32)
    aW2 = np.asarray(aW2, np.float32)
    ab2 = np.asarray(ab2, np.float32)
    # aW1 [512, 512] -> [128, 4kc * 512] (K-chunk kc at cols kc*512)
    aW1_p = aW1.reshape(4, 128, D).transpose(1, 0, 2).reshape(128, 4 * D)
    ab1_p = ab1.reshape(4, 128).T.copy()             # [128, 4]
    aW2_p = aW2.reshape(4, 128).T.copy()             # [128, 4]
    ab2_p = ab2.reshape(1, 1)
    common = {
        "aW1": aW1_p.astype(BF),
        "ab1": np.ascontiguousarray(ab1_p),
        "aW2": aW2_p.astype(BF),
        "ab2": np.ascontiguousarray(ab2_p),
    }
    in_maps = []
    for c in range(N_CORES):
        b, half = c // 2, c % 2
        nvalid = R_EVEN if half == 0 else R_ODD
        lo = 0 if half == 0 else R_EVEN
        xs = np.zeros((R_PAD, D), np.float32)
        xs[:nvalid] = x[b, lo:lo + nvalid]
        xs = xs.astype(BF)
        # natural block-major: [NBLK, 128, NSUB, D]
        xn = np.ascontiguousarray(
            xs.reshape(NBLK, NSUB, 128, D).transpose(0, 2, 1, 3))
        # feature-major block-major: [NBLK, 128, 4, BLK]
        xt = np.ascontiguousarray(
            xs.reshape(NBLK, BLK, 4, 128).transpose(0, 3, 2, 1))
        in_maps.append({"xn": xn, "xt": xt, **common})
    return in_maps


def _lrelu(x, s=0.01):
    return np.where(x >= 0, x, s * x)


def _graph_norm(x, w, b, ms):
    mean = x.mean(axis=0)
    out = x - mean * ms
    var = (out * out).mean(axis=0)
    return w * out / np.sqrt(var + EPS_GN) + b


def _hypergraph_conv_dense(x, he_attr, S, W, att, bias):
    """Dense form of PyG HypergraphConv (heads=1, attention) on the kNN
    hypergraph. S[i, j] = 1 iff j in nbr[i]; hyperedge i has the K nbrs of
    node i. Edge (dst=i, src=j) exists iff S[i,j]=1."""
    xw = x @ W                       # [256, F]
    hw = he_attr @ W                 # [256, F]
    d = att.shape[0] // 2
    p = xw @ att[:d]                 # [256] (src term)
    q = hw @ att[d:]                 # [256] (dst term)
    A = _lrelu(q[:, None] + p[None, :], 0.2)     # [i, j]
    mask = S > 0
    neg = np.where(mask, A, -np.inf)
    col_max = neg.max(axis=0)                    # per src node j
    col_max = np.where(np.isfinite(col_max), col_max, 0.0)
    E = np.where(mask, np.exp(A - col_max[None, :]), 0.0)
    colsum = E.sum(axis=0)
    colsum = np.where(colsum > 0, colsum, 1.0)
    alpha = E / colsum[None, :]
    deg = S.sum(axis=0)                          # node degree D (as src)
    Dinv = np.where(deg > 0, 1.0 / deg, 0.0)
    edge_feat = (alpha / K) @ xw                 # node -> hyperedge
    out = Dinv[:, None] * (alpha.T @ edge_feat)  # hyperedge -> node
    return out + bias


def _graph_stage(M, rehearsal, dW1, db1, dW2, db2,
                 h1W, h1att, h1b, h2W, h2att, h2b,
                 n1w, n1b, n1ms, n2w, n2b, n2ms,
                 f1W, f1b, f2W, f2b, gW1, gb1, gW2, gb2, cW, cb):
    x_concat = np.concatenate([M, rehearsal], axis=0)[:NUM_NODE]
    xf = _lrelu(_lrelu(x_concat @ dW1 + db1) @ dW2 + db2)
    norms = np.maximum(np.linalg.norm(xf, axis=1, keepdims=True), 1e-12)
    xn = xf / norms
    sim = xn @ xn.T
    # top-K per row via threshold on the 8th largest value
    kth = np.partition(sim, NUM_NODE - K, axis=1)[:, NUM_NODE - K]
    S = (sim >= kth[:, None]).astype(np.float32)
    edge_attr = (S @ xf) / K

    g1 = _hypergraph_conv_dense(xf, edge_attr, S, h1W, h1att, h1b)
    g1 = _lrelu(_graph_norm(g1, n1w, n1b, n1ms))
    out1 = _lrelu(g1 @ f1W + f1b)
    g2 = _hypergraph_conv_dense(g1, edge_attr, S, h2W, h2att, h2b)
    g2 = _lrelu(_graph_norm(g2, n2w, n2b, n2ms))
    out2 = _lrelu(g2 @ f2W + f2b)

    out = np.concatenate([xf, out1, out2], axis=1)   # [256, 1024]
    s = np.maximum(out.T @ gW1 + gb1, 0.0) @ gW2 + gb2
    s = 1.0 / (1.0 + np.exp(-s))
    s = s[:, 0] - np.mean(s)
    logits = (out * s[None, :]) @ cW + cb
    return logits


def kernel(**inputs):
    inp = {k: np.asarray(v) for k, v in inputs.items()}
    x = inp["x"].astype(np.float32)

    nc = _get_mil_program()
    in_maps = _make_in_maps(x, inp["aW1"], inp["ab1"], inp["aW2"], inp["ab2"])
    res = bass_utils.run_bass_kernel_spmd(nc, in_maps, core_ids=list(range(N_CORES)))

    m = np.stack([res.results[c]["m_part"][0] for c in range(N_CORES)])  # [8, 512]
    s = np.array([
        res.results[c]["e_part"][0, :(R_EVEN if c % 2 == 0 else R_ODD)].sum()
        for c in range(N_CORES)
    ])
    M = np.stack([(m[2 * b] + m[2 * b + 1]) / (s[2 * b] + s[2 * b + 1])
                  for b in range(B)]).astype(np.float32)                 # [4, 512]

    logits_mlp = (M @ inp["bagW"] + inp["bagB"]).astype(np.float32)

    gkeys = ["rehearsal", "dW1", "db1", "dW2", "db2",
             "h1W", "h1att", "h1b", "h2W", "h2att", "h2b",
             "n1w", "n1b", "n1ms", "n2w", "n2b", "n2ms",
             "f1W", "f1b", "f2W", "f2b", "gW1", "gb1", "gW2", "gb2", "cW", "cb"]
    logits = _graph_stage(M, *[inp[k].astype(np.float32) for k in gkeys])
    logits_graph = logits[:B].astype(np.float32)
    return logits_mlp, logits_graph
